# revision 9
# baseline (speedup 1.0000x reference)
"""Locally-connected layer (unshared 3x3 conv, torch-unfold semantics) on 8 trn2 cores.

out[b,o,y,x] = sum_{c,i,j} weight[o, c*9+i*3+j, y*32+x] * xpad[b, c, y+i, x+j] + bias[o, l]

Sharding: spatial over L — core r owns image rows [4r, 4r+4) (128 pixels).

v2: all-bf16 datapath (tolerance is 2e-2; bf16 contributes ~3e-3).
  - 9 (or fewer at the j-edges) K=64 matmuls per pixel: x patch stationary
    [C, B], per-pixel weights moving [C, O].  No shifted x copies — HBM
    traffic is the bottleneck (weights 9.4 MB/core bf16, used exactly once).
  - Host pre-packs all DRAM tensors into exact SBUF layouts: every DMA
    descriptor moves >=512B contiguous per partition (full DMA rate).
  - x stored unpadded [slab_row, C, W*B], one tile+DMA per slab row; taps
    that would read the zero pad columns are skipped.
  - Weights stream on SP in 8px chunks (one PSUM bank / one accumulation
    group per pixel, drained by a DVE copy).  Matmuls stay PIXEL-MAJOR:
    interleaving pixels' open PSUM groups corrupts results on hardware.
  - Queue topology (each choice measured): weights + late x rows + the
    merged final output on SP; mid-stream outputs on gpsimd/SWDGE (its sem
    pool is separate from the 7 shared HWDGE DMA sems, so copy-gated outs
    cannot stall weight issues); last row's earlier outputs on Act; x rows
    0/2 on SP and 1 on Act at the head; late x rows emitted after each
    row's last weight chunk so they slot mid-stream without delaying it.
  - Last row split 8/8/8/6/2 px: the 2px copy gates the merged final DMA
    as late as possible, minimizing the end-of-kernel drain chain.
  - PE p-state warm-up: dummy matmuls on scratch SBUF ramp the tensor
    clock to 2.4 GHz while the first chunks are still in flight on DMA.
"""

import numpy as np
import ml_dtypes

B, C, O, H, W, KS = 64, 64, 64, 32, 32, 3
L = H * W
NCORES = 8
RPC = H // NCORES            # image rows per core = 4
HALO = RPC + 2               # 6 slab rows

# pixel-range chunks per y row: full rows mid-stream, short chunks at the end
# compute chunks (PSUM-group granularity)
CHUNKS = {
    0: [(0, 8), (8, 16), (16, 24), (24, 32)],
    1: [(0, 8), (8, 16), (16, 24), (24, 32)],
    2: [(0, 8), (8, 16), (16, 24), (24, 32)],
    3: [(0, 8), (8, 16), (16, 24), (24, 30), (30, 32)],
}
# weight-DMA granularity (>=4px: smaller transfers hit the <512B descriptor
# penalty); the last row's short compute chunks share an 8px weight tile
WGROUPS = {
    0: CHUNKS[0],
    1: CHUNKS[1],
    2: CHUNKS[2],
    3: [(0, 8), (8, 16), (16, 24), (24, 32)],
}
N_WARM = 115         # PE clock warm-up dummies before the first real matmul
# virtual-time floors (us) pinning late x rows behind the early weight stream
XROW_WAIT_US = {3: 8.0, 4: 15.0, 5: 22.0}


_CACHE = {}


def _taps(x):
    """(i, j) taps for intra-row pixel x, skipping zero-pad columns."""
    return [
        (i, j)
        for i in range(KS)
        for j in range(KS)
        if 0 <= x + j - 1 < W
    ]


def _build_nc():
    import concourse.bass as bass
    import concourse.bacc as bacc
    import concourse.tile as tile
    from concourse import mybir

    bf16 = mybir.dt.bfloat16
    f32 = mybir.dt.float32
    nc = bacc.Bacc(
        "TRN2", target_bir_lowering=False, debug=False, num_devices=NCORES
    )
    x_d = nc.dram_tensor("x", [HALO, C, W * B], bf16, kind="ExternalInput")
    w_d = nc.dram_tensor("w", [RPC, C, KS * KS, W, O], bf16, kind="ExternalInput")
    o_d = nc.dram_tensor("out", [B, RPC, W * O], bf16, kind="ExternalOutput")

    with tile.TileContext(nc) as tc:
        with (
            tc.tile_pool(name="xr0", bufs=1) as xpool0,
            tc.tile_pool(name="xr1", bufs=1) as xpool1,
            tc.tile_pool(name="xr2", bufs=1) as xpool2,
            tc.tile_pool(name="xr3", bufs=1) as xpool3,
            tc.tile_pool(name="xr4", bufs=1) as xpool4,
            tc.tile_pool(name="xr5", bufs=1) as xpool5,
            tc.tile_pool(name="dum", bufs=1) as dpool,
            tc.tile_pool(name="wt", bufs=12) as wpool,
            tc.tile_pool(name="orow", bufs=2) as opool,
            tc.tile_pool(name="ps", bufs=4, space=bass.MemorySpace.PSUM) as pspool,
            tc.tile_pool(name="psd", bufs=1, space=bass.MemorySpace.PSUM) as psdpool,
        ):
            # PE warm-up scratch: written once by DVE, read by dummy matmuls
            dum = dpool.tile([C, 2 * B], bf16)
            nc.vector.memset(dum[:], 0.0)
            psd = psdpool.tile([B, B], f32)

            def warm(n):
                for _ in range(n):
                    nc.tensor.matmul(
                        psd[:], dum[:, 0:B], dum[:, B : 2 * B],
                        start=True, stop=True,
                    )

            warm(N_WARM)

            # one tile per slab row (separate pools: a shared tile would make
            # later row-writes WAR-wait on all earlier readers, stalling the
            # SP queue).  Layout [c, (xcol b)]; b contiguous per column.
            xpools = [xpool0, xpool1, xpool2, xpool3, xpool4, xpool5]
            xrow = [
                p.tile([C, W * B], bf16, name=f"xr{i}_t")
                for i, p in enumerate(xpools)
            ]
            xvr = [t[:].rearrange("c (w b) -> c w b", b=B) for t in xrow]
            nc.sync.dma_start(xrow[0][:], x_d[0])
            nc.scalar.dma_start(xrow[1][:], x_d[1])
            nc.sync.dma_start(xrow[2][:], x_d[2])

            for y in range(RPC):
                orow = opool.tile([B, W * O], bf16)
                # weight DMAs for this row
                wtiles = []
                for gi, (g0, g1) in enumerate(WGROUPS[y]):
                    wt = wpool.tile([C, KS * KS, g1 - g0, O], bf16, name="wc")
                    nc.sync.dma_start(wt[:], w_d[y, :, :, g0:g1])
                    wtiles.append((g0, g1, wt))
                    if gi == 3 and y + 3 < HALO:
                        # slab row y+3 (first needed by output row y+1) rides
                        # SP mid-stream: equally-ready SP DMAs keep emission
                        # order, so it slots after this row's second weight
                        # chunk instead of hoisting ahead of w00.  (Safe only
                        # because outputs live on SWDGE — an output DMA in
                        # the 7-sem HWDGE rotation would stall this issue.)
                        nc.sync.dma_start(xrow[y + 3][:], x_d[y + 3])

                for ci, (x0, x1) in enumerate(CHUNKS[y]):
                    npix = x1 - x0
                    g0, g1, wt = next(
                        g for g in wtiles if g[0] <= x0 and x1 <= g[1]
                    )
                    ps = pspool.tile([B, npix * O], f32)

                    def mm(xi, i, j):
                        x = x0 + xi
                        taps = _taps(x)
                        nc.tensor.matmul(
                            ps[:, xi * O : (xi + 1) * O],
                            xvr[y + i][:, x + j - 1, :],   # [C, B] stationary
                            wt[:, i * KS + j, x - g0, :],  # [C, O] moving
                            start=(i, j) == taps[0], stop=(i, j) == taps[-1],
                        )

                    # pixel-major only: a pixel's PSUM accumulation group
                    # must stay contiguous on the PE — interleaving groups
                    # (tap-major) corrupts results on real hardware
                    for xi in range(npix):
                        for (i, j) in _taps(x0 + xi):
                            mm(xi, i, j)

                    dst = orow[:, x0 * O : x1 * O]
                    nc.vector.tensor_copy(dst, ps[:])
                    # all outputs except the merged final ride gpsimd/SWDGE
                    # (separate sem pool; can't clog the HWDGE window); the
                    # merged final two chunks use SP, whose SEQ is then free
                    # of prior copy-gated holds at the tail
                    lastrow = y == RPC - 1
                    if lastrow and ci == len(CHUNKS[y]) - 2:
                        continue
                    if lastrow and ci == len(CHUNKS[y]) - 1:
                        xm = CHUNKS[y][-2][0]
                        nc.sync.dma_start(
                            o_d[:, y, xm * O : x1 * O], orow[:, xm * O : x1 * O]
                        )
                    elif lastrow:
                        # last row's earlier outputs on Act (idle since the
                        # head; HWDGE issue beats SWDGE desc-gen by ~0.4us and
                        # this late there is no downstream rotation victim)
                        nc.scalar.dma_start(o_d[:, y, x0 * O : x1 * O], dst)
                    else:
                        nc.gpsimd.dma_start(o_d[:, y, x0 * O : x1 * O], dst)
    nc.compile()
    return nc


def _get_nc():
    if "nc" not in _CACHE:
        _CACHE["nc"] = _build_nc()
    return _CACHE["nc"]


def _shard_inputs(x, weight):
    bf16 = ml_dtypes.bfloat16
    # [r, y, c, t, x, o] from weight [o, (c t), ((r y) x)]
    wpk = np.ascontiguousarray(
        weight.reshape(O, C, KS * KS, NCORES, RPC, W)
        .transpose(3, 4, 1, 2, 5, 0)
        .astype(bf16)
    )
    # x slab rows for core r are padded rows [4r, 4r+6); W kept unpadded
    # (pad-column taps are skipped on device).
    xrows = np.pad(x, ((0, 0), (0, 0), (1, 1), (0, 0)))  # [B, C, H+2, W]
    in_maps = []
    for r in range(NCORES):
        # [slab_row, c, w, b]
        xs = np.ascontiguousarray(
            xrows[:, :, RPC * r : RPC * r + HALO, :]
            .transpose(2, 1, 3, 0)
            .reshape(HALO, C, W * B)
            .astype(bf16)
        )
        in_maps.append({"x": xs, "w": wpk[r]})
    return in_maps


def kernel(x, weight, bias, _trace=False, _trace_kwargs=None):
    from concourse.bass_utils import run_bass_kernel_spmd

    x = np.asarray(x, dtype=np.float32)
    weight = np.asarray(weight, dtype=np.float32)
    bias = np.asarray(bias, dtype=np.float32)

    nc = _get_nc()
    in_maps = _shard_inputs(x, weight)
    res = run_bass_kernel_spmd(
        nc, in_maps, list(range(NCORES)),
        trace=_trace, **(_trace_kwargs or {}),
    )
    # per core: [B, RPC, W*O] bf16, (y, x, o) -> [B, O, RPC, W]
    parts = [
        np.asarray(res.results[r]["out"])
        .astype(np.float32)
        .reshape(B, RPC, W, O)
        .transpose(0, 3, 1, 2)
        for r in range(NCORES)
    ]
    out = np.concatenate(parts, axis=2)
    if np.any(bias):
        out = out + bias.reshape(1, O, H, W)
    if _trace:
        _CACHE["last_result"] = res
    return np.ascontiguousarray(out.astype(np.float32))
